# revision 9
# baseline (speedup 1.0000x reference)
"""Trainium2 kernel for nn_LocalEncoder (BLT-style local encoder).

Key structural insight: every per-token quantity (boundary logit z, rmsnorm
scale, q/k/v projections) depends only on the token ID (vocab=260), so all
dense math runs on the 260-row vocab tables instead of 16384 token rows.
Top-k boundary selection ties (same token id => bit-identical z in the fp32
reference) are broken by index, reproduced exactly on the host.

Pipeline:
  Kernel A (8 cores, DF split 8x384): zv partials = w2_slice @ silu(w1_slice @ embT)
  Host:     zv -> per-row boundary selection (stable by (-z, idx)) -> pos/pid/qtok
  Kernel B (8 cores = 4 seqs x 2 query-halves): one-hot gathers of vocab
            q/k/v tables per token, per-token scores + exp on DVE/ACT,
            block-diagonal softmax via one-hot scatter matmuls, wo proj.
"""

import os
import numpy as np

import concourse.bass as bass
import concourse.bacc as bacc
import concourse.mybir as mybir
from concourse.tile import TileContext
from concourse.alu_op_type import AluOpType
from concourse.bass_utils import run_bass_kernel_spmd
from concourse import masks

F32 = mybir.dt.float32
F32R = mybir.dt.float32r
AFT = mybir.ActivationFunctionType
AX = mybir.AxisListType

B, L, D, V, K, H, HD = 4, 4096, 768, 260, 512, 12, 64
DF = 4 * D
VP = 384          # vocab padded to 3 partition chunks
RMS_EPS = 1e-5
NCORES = 8
FSL = DF // NCORES  # 384 f-rows per core in kernel A

_cache = {}


# --------------------------------------------------------------------------- #
# Kernel A: per-core partial zv over a DF slice (fp32 matmuls for precision)
# --------------------------------------------------------------------------- #
def build_kernel_a():
    nc = bacc.Bacc("TRN2", target_bir_lowering=False, debug=False)
    embT_d = nc.dram_tensor("embT", [D, V], F32, kind="ExternalInput")
    w1T_d = nc.dram_tensor("w1T", [D, FSL], F32, kind="ExternalInput")
    b1_d = nc.dram_tensor("b1s", [FSL], F32, kind="ExternalInput")
    w2_d = nc.dram_tensor("w2s", [FSL], F32, kind="ExternalInput")
    zp_d = nc.dram_tensor("zp", [1, V], F32, kind="ExternalOutput")

    with TileContext(nc) as tc:
        with (
            tc.tile_pool(name="sb", bufs=1) as sb,
            tc.tile_pool(name="ps", bufs=2, space="PSUM") as ps,
        ):
            embT = [sb.tile([128, V], F32, tag=f"embT{d}", name=f"embT{d}") for d in range(6)]
            w1T = [sb.tile([128, FSL], F32, tag=f"w1T{d}", name=f"w1T{d}") for d in range(6)]
            for d in range(6):
                nc.sync.dma_start(embT[d][:, :], embT_d[128 * d:128 * (d + 1), :])
                nc.sync.dma_start(w1T[d][:, :], w1T_d[128 * d:128 * (d + 1), :])
            b1c = sb.tile([128, 3], F32, tag="b1c")
            w2c = sb.tile([128, 3], F32, tag="w2c")
            nc.sync.dma_start(b1c[:, :], b1_d.rearrange("(i p) -> p i", p=128))
            nc.sync.dma_start(w2c[:, :], w2_d.rearrange("(i p) -> p i", p=128))

            zp_ps = ps.tile([1, V], F32, tag="zp")
            for fi in range(3):
                y1p = ps.tile([128, V], F32, tag="y1")
                for d in range(6):
                    nc.tensor.matmul(
                        y1p[:, :], w1T[d][:, 128 * fi:128 * (fi + 1)], embT[d][:, :],
                        start=(d == 0), stop=(d == 5),
                    )
                y1b = sb.tile([128, V], F32, tag="y1b")
                nc.vector.tensor_scalar(y1b[:, :], y1p[:, :], b1c[:, fi:fi + 1],
                                        None, AluOpType.add)
                sig = sb.tile([128, V], F32, tag="sig")
                nc.scalar.activation(sig[:, :], y1b[:, :], AFT.Sigmoid)
                y1s = sb.tile([128, V], F32, tag="y1s")
                nc.vector.tensor_tensor(y1s[:, :], y1b[:, :], sig[:, :],
                                        AluOpType.mult)
                nc.tensor.matmul(zp_ps[:, :], w2c[:, fi:fi + 1], y1s[:, :],
                                 start=(fi == 0), stop=(fi == 2))
            zp_s = sb.tile([1, V], F32, tag="zps")
            nc.vector.tensor_copy(zp_s[:, :], zp_ps[:, :])
            nc.sync.dma_start(zp_d[:, :], zp_s[:, :])
    nc.compile()
    return nc


def run_kernel_a(inputs):
    if "A" not in _cache:
        _cache["A"] = build_kernel_a()
    nc = _cache["A"]
    embT = np.ascontiguousarray(inputs["embed_W"].astype(np.float32).T)
    w1 = inputs["bp_w1"].astype(np.float32)
    b1 = inputs["bp_b1"].astype(np.float32)
    w2 = inputs["bp_w2"].astype(np.float32)[0]
    in_maps = []
    for c in range(NCORES):
        sl = slice(c * FSL, (c + 1) * FSL)
        in_maps.append({
            "embT": embT,
            "w1T": np.ascontiguousarray(w1[sl].T),
            "b1s": np.ascontiguousarray(b1[sl]),
            "w2s": np.ascontiguousarray(w2[sl]),
        })
    res = run_bass_kernel_spmd(nc, in_maps, list(range(NCORES)),
                               trace=os.environ.get("KERNEL_TRACE") == "1")
    _cache["tA"] = res.exec_time_ns
    zv = np.zeros(V, np.float64)
    for c in range(NCORES):
        zv += res.results[c]["zp"][0].astype(np.float64)
    zv += inputs["bp_b2"].astype(np.float64)[0]
    return zv.astype(np.float32)


# --------------------------------------------------------------------------- #
# Host boundary logic
# --------------------------------------------------------------------------- #
def boundary_plan(zv, tokens):
    """Reproduce reference top-k (stable ties by index) + patch structure."""
    zt = zv[tokens]  # [B, L]
    pos = np.zeros((B, K), np.int64)
    for b in range(B):
        key = zt[b].astype(np.float64).copy()
        key[0] = np.inf  # position 0 forced boundary (logprob set to 0 = max)
        order = np.lexsort((np.arange(L), -key))
        pos[b] = np.sort(order[:K])
    pid = (pos[:, None, :] <= np.arange(L)[None, :, None]).sum(-1) - 1  # [B, L]
    qtok = np.take_along_axis(tokens, np.take_along_axis(pos, pid, 1), 1)  # [B, L]
    return pos, pid, qtok


# --------------------------------------------------------------------------- #
# Kernel B: sparse cross-attention via vocab tables + one-hot matmuls
# --------------------------------------------------------------------------- #
def build_kernel_b(jobs):
    """jobs: per-core dict with j0 and per-jc tchunk ranges (python ints ->
    data-dependent instruction stream; same NEFF runs on all 8 cores with the
    max structure, masking handles core differences).  To keep one NEFF for
    all cores, we use the UNION structure: every core runs the same tchunk
    count per jc slot; tchunk indices and j0 are per-core DATA (iota bases
    must be static though) -- so instead we compile per-core variants only if
    structure differs.  Simpler: compile ONE program parameterized by the max
    chunk counts; per-core tchunk starts enter via DRAM-provided pid/tok/qtok
    columns (already per-core rebased by host).
    """
    n0, n1 = jobs["n0"], jobs["n1"]  # tchunks for jc0 / jc1 (uniform, padded)
    nc = bacc.Bacc("TRN2", target_bir_lowering=False, debug=False)

    # vocab tables
    emb_d = nc.dram_tensor("emb", [VP, D], F32, kind="ExternalInput")
    embT_d = nc.dram_tensor("embT", [D, VP], F32R, kind="ExternalInput")
    wqT_d = nc.dram_tensor("wqT", [D, D], F32R, kind="ExternalInput")
    wkT_d = nc.dram_tensor("wkT", [D, D], F32R, kind="ExternalInput")
    wvT_d = nc.dram_tensor("wvT", [D, D], F32R, kind="ExternalInput")
    woT_d = nc.dram_tensor("woT", [D, D], F32R, kind="ExternalInput")
    # per-core token structure, already sliced/padded by host:
    # rows: [1, NT*128] token ids / boundary-token ids (f32), NT = n0+n1
    NT = n0 + n1
    tqr_d = nc.dram_tensor("tqr", [1, NT * 256], F32R, kind="ExternalInput")
    pidc_d = nc.dram_tensor("pidc", [NT * 128], F32, kind="ExternalInput")
    out_d = nc.dram_tensor("out", [256, D], F32, kind="ExternalOutput")

    with TileContext(nc) as tc:
        with (
            tc.tile_pool(name="sb", bufs=1) as sb,
            tc.tile_pool(name="wk", bufs=1) as wkp,
            tc.tile_pool(name="ps", bufs=1, space="PSUM") as ps,
            tc.tile_pool(name="acc", bufs=1, space="PSUM") as accp,
        ):
            # ---- global small tiles ----
            ones_f = sb.tile([1, 128], F32, tag="onesf")
            nc.gpsimd.memset(ones_f[:, :], 1.0)
            ones_r = sb.tile([1, 128], F32R, tag="onesr")
            nc.vector.tensor_copy(ones_r[:, :], ones_f[:, :])
            ident = sb.tile([128, 128], F32, tag="ident")
            masks.make_identity(nc, ident[:, :])
            iotav = sb.tile([128, 3], F32, tag="iotav")  # col p+0/128/256
            for vc in range(3):
                nc.gpsimd.iota(iotav[:, vc:vc + 1], [[0, 1]], base=128 * vc,
                               channel_multiplier=1,
                               allow_small_or_imprecise_dtypes=True)
            iotaj = [sb.tile([128, 128], F32, tag=f"iotaj{jc}", name=f"iotaj{jc}") for jc in range(2)]
            for jc in range(2):
                # value = j0 + jc*128 + f ; j0 enters via host-rebased pid
                nc.gpsimd.iota(iotaj[jc][:, :], [[1, 128]], base=128 * jc,
                               channel_multiplier=0,
                               allow_small_or_imprecise_dtypes=True)

            # ---- load weights / tables ----
            emb = [sb.tile([128, D], F32, tag=f"emb{v}", name=f"emb{v}") for v in range(3)]
            for v in range(3):
                nc.sync.dma_start(emb[v][:, :], emb_d[128 * v:128 * (v + 1), :])
            embT = [sb.tile([128, VP], F32R, tag=f"embT{d}", name=f"embTb{d}") for d in range(6)]
            for d in range(6):
                nc.sync.dma_start(embT[d][:, :], embT_d[128 * d:128 * (d + 1), :])
            wts = {}
            for nm, dd in (("wq", wqT_d), ("wk", wkT_d), ("wv", wvT_d), ("wo", woT_d)):
                wts[nm] = [wkp.tile([128, D], F32R, tag=f"{nm}{d}", name=f"{nm}_{d}") for d in range(6)]
                for d in range(6):
                    nc.sync.dma_start(wts[nm][d][:, :], dd[128 * d:128 * (d + 1), :])

            # per-tchunk structure columns
            pidc = sb.tile([128, NT], F32, tag="pidc")
            nc.sync.dma_start(pidc[:, :], pidc_d.rearrange("(i p) -> p i", p=128))

            # ---- rmsnorm scales: rv (k/v), rv8 (q, includes /8) ----
            msq = sb.tile([128, 3], F32, tag="msq")
            sqjunk = sb.tile([128, D], F32, tag="sqjunk")
            for v in range(3):
                nc.scalar.activation(sqjunk[:, :], emb[v][:, :], AFT.Square,
                                     accum_out=msq[:, v:v + 1])
            # rv = (msq/768 + eps)^-1/2 = exp(-0.5*ln(msq/768 + eps))
            epsc = sb.tile([128, 1], F32, tag="epsc")
            nc.gpsimd.memset(epsc[:, :], RMS_EPS)
            lnv = sb.tile([128, 3], F32, tag="lnv")
            nc.scalar.activation(lnv[:, :], msq[:, :], AFT.Ln,
                                 scale=1.0 / D, bias=epsc[:, :1])
            rv = sb.tile([128, 3], F32, tag="rv")
            nc.scalar.activation(rv[:, :], lnv[:, :], AFT.Exp, scale=-0.5)
            rv8 = sb.tile([128, 3], F32, tag="rv8")
            nc.vector.tensor_scalar(rv8[:, :], rv[:, :], 0.125, None, AluOpType.mult)

            # ---- vocab tables q_s / k_n / v_n [3][128, D] f32r ----
            tabs = {}
            for nm, wname, scl in (("q", "wq", rv8), ("k", "wk", rv), ("v", "wv", rv)):
                tabs[nm] = []
                for v in range(3):
                    tp = ps.tile([128, D], F32, tag="qg", name="tp")
                    for d in range(6):
                        nc.tensor.matmul(
                            tp[:, :512], embT[d][:, 128 * v:128 * (v + 1)],
                            wts[wname][d][:, :512], start=(d == 0), stop=(d == 5))
                        nc.tensor.matmul(
                            tp[:, 512:], embT[d][:, 128 * v:128 * (v + 1)],
                            wts[wname][d][:, 512:], start=(d == 0), stop=(d == 5))
                    ts_ = sb.tile([128, D], F32R, tag=f"tab{nm}{v}")
                    nc.vector.tensor_scalar(ts_[:, :], tp[:, :], scl[:, v:v + 1],
                                            None, AluOpType.mult)
                    tabs[nm].append(ts_)

            # ---- main loop: two query chunks ----
            for jc in range(2):
                ntc = n0 if jc == 0 else n1
                base = 0 if jc == 0 else n0
                acc = accp.tile([128, 1536], F32, tag="acc", name="acc")
                for i in range(ntc):
                    tci = base + i
                    # broadcast token+qtok rows across partitions (one matmul)
                    tq_s = sb.tile([1, 256], F32R, tag="tokslice", name="tq_s")
                    nc.sync.dma_start(tq_s[:, :], tqr_d[:, 256 * tci:256 * (tci + 1)])
                    btok2 = ps.tile([128, 256], F32, tag="btok", name="btok2")
                    nc.tensor.matmul(btok2[:, :], ones_r[:, :], tq_s[:, :],
                                     start=True, stop=True)
                    btok = btok2[:, :128]
                    bqtok = btok2[:, 128:]
                    ohk = []
                    ohq = []
                    for v in range(3):
                        o1 = sb.tile([128, 128], F32R, tag=f"ohk{v}")
                        nc.vector.tensor_scalar(o1[:, :], btok,
                                                iotav[:, v:v + 1], None,
                                                AluOpType.is_equal)
                        ohk.append(o1)
                        o2 = sb.tile([128, 128], F32R, tag=f"ohq{v}")
                        nc.vector.tensor_scalar(o2[:, :], bqtok,
                                                iotav[:, v:v + 1], None,
                                                AluOpType.is_equal)
                        ohq.append(o2)
                    # gathers: qg/kg/vg [t,768]
                    qg = ps.tile([128, D], F32, tag="qg")
                    kg = ps.tile([128, D], F32, tag="kg")
                    for v in range(3):
                        nc.tensor.matmul(qg[:, :512], ohq[v][:, :],
                                         tabs["q"][v][:, :512],
                                         start=(v == 0), stop=(v == 2))
                        nc.tensor.matmul(qg[:, 512:], ohq[v][:, :],
                                         tabs["q"][v][:, 512:],
                                         start=(v == 0), stop=(v == 2))
                        nc.tensor.matmul(kg[:, :512], ohk[v][:, :],
                                         tabs["k"][v][:, :512],
                                         start=(v == 0), stop=(v == 2))
                        nc.tensor.matmul(kg[:, 512:], ohk[v][:, :],
                                         tabs["k"][v][:, 512:],
                                         start=(v == 0), stop=(v == 2))
                    # scores + exp
                    kgs = sb.tile([128, D], F32, tag="kgs")
                    nc.scalar.copy(kgs[:, :], kg[:, :])
                    prod = sb.tile([128, D], F32, tag="prod")
                    nc.vector.tensor_tensor(prod[:, :], qg[:, :], kgs[:, :],
                                            AluOpType.mult)
                    s12 = sb.tile([128, H], F32, tag="s12")
                    nc.vector.tensor_reduce(
                        ap3(s12, H, 1), ap3(prod, H, HD), AX.X, AluOpType.add)
                    e12f = sb.tile([128, H], F32, tag="e12f")
                    nc.scalar.activation(e12f[:, :], s12[:, :], AFT.Exp)
                    e12 = sb.tile([128, H], F32R, tag="e12")
                    nc.vector.tensor_copy(e12[:, :], e12f[:, :])
                    # value gather (reuses qg slot) and weight
                    vg = ps.tile([128, D], F32, tag="qg")
                    for v in range(3):
                        nc.tensor.matmul(vg[:, :512], ohk[v][:, :],
                                         tabs["v"][v][:, :512],
                                         start=(v == 0), stop=(v == 2))
                        nc.tensor.matmul(vg[:, 512:], ohk[v][:, :],
                                         tabs["v"][v][:, 512:],
                                         start=(v == 0), stop=(v == 2))
                    wv = sb.tile([128, D], F32R, tag="wv")
                    nc.vector.tensor_tensor(ap3(wv, H, HD),
                                            bcast3(e12f, H, HD),
                                            ap3(vg, H, HD), AluOpType.mult)
                    # membership MT [t, j] and scatter
                    mt = sb.tile([128, 128], F32R, tag="mt")
                    nc.vector.tensor_scalar(mt[:, :], iotaj[jc][:, :],
                                            pidc[:, tci:tci + 1], None,
                                            AluOpType.is_equal)
                    nc.tensor.matmul(acc[:, :512], mt[:, :], wv[:, :512],
                                     start=(i == 0), stop=(i == ntc - 1))
                    nc.tensor.matmul(acc[:, 512:768], mt[:, :], wv[:, 512:],
                                     start=(i == 0), stop=(i == ntc - 1))
                    nc.tensor.matmul(acc[:, 1024:1036], mt[:, :], e12[:, :],
                                     start=(i == 0), stop=(i == ntc - 1))
                # ---- finalize jc ----
                lnz = sb.tile([128, H], F32, tag="lnz")
                nc.scalar.activation(lnz[:, :], acc[:, 1024:1036], AFT.Ln)
                zrec = sb.tile([128, H], F32, tag="zrec")
                nc.scalar.activation(zrec[:, :], lnz[:, :], AFT.Exp, scale=-1.0)
                pr = sb.tile([128, D], F32, tag="pr")
                nc.vector.tensor_tensor(ap3(pr, H, HD), bcast3(zrec, H, HD),
                                        ap3(acc, H, HD, width=780), AluOpType.mult)
                fin = ps.tile([128, D], F32, tag="kg", name="fin")
                for d in range(6):
                    trp = ps.tile([128, 128], F32, tag="btok")
                    nc.tensor.transpose(trp[:, :], pr[:, 128 * d:128 * (d + 1)],
                                        ident[:, :])
                    trs = sb.tile([128, 128], F32R, tag="trs")
                    nc.vector.tensor_copy(trs[:, :], trp[:, :])
                    nc.tensor.matmul(fin[:, :512], trs[:, :], wts["wo"][d][:, :512],
                                     start=(d == 0), stop=(d == 5))
                    nc.tensor.matmul(fin[:, 512:], trs[:, :], wts["wo"][d][:, 512:],
                                     start=(d == 0), stop=(d == 5))
                fin_s = sb.tile([128, D], F32, tag="fins")
                nc.vector.tensor_copy(fin_s[:, :], fin[:, :])
                nc.sync.dma_start(out_d[128 * jc:128 * (jc + 1), :], fin_s[:, :])
    nc.compile()
    return nc


def ap3(tile, n, w, width=None):
    """[128, n*w] tile viewed as [128, n, w] (first n*w cols)."""
    p = tile.ap[0] if hasattr(tile, "ap") else None
    t = tile[:, :]
    ps, fs = t.ap[0], t.ap[1]
    return bass.AP(t.tensor, t.offset, [list(ps), [fs[0] * w, n], [fs[0], w]])


def bcast3(tile, n, w):
    """[128, n] tile broadcast to [128, n, w] via 0-stride inner dim."""
    t = tile[:, :]
    ps, fs = t.ap[0], t.ap[1]
    return bass.AP(t.tensor, t.offset, [list(ps), [fs[0], n], [0, w]])


# --------------------------------------------------------------------------- #
# top-level
# --------------------------------------------------------------------------- #
def kernel(tokens, embed_W, bp_w1, bp_b1, bp_w2, bp_b2, wq, wk, wv, wo,
           qnorm_w, kvnorm_w, k_patches):
    tokens = np.asarray(tokens).astype(np.int64)
    inputs = dict(tokens=tokens, embed_W=embed_W, bp_w1=bp_w1, bp_b1=bp_b1,
                  bp_w2=bp_w2, bp_b2=bp_b2)
    zv = run_kernel_a(inputs)
    pos, pid, qtok = boundary_plan(zv, tokens)

    # per-core job structure: core = 2*b + half; queries [half*256, half*256+256)
    cores = []
    for b in range(B):
        for half in range(2):
            j0 = half * 256
            ends = [pos[b, j0 + 128] if j0 + 128 < K else L,
                    pos[b, j0 + 256] if j0 + 256 < K else L]
            starts = [pos[b, j0], pos[b, j0 + 128] if j0 + 128 < K else L]
            tcs = []
            for jc in range(2):
                lo, hi = int(starts[jc]) // 128, -(-int(ends[jc]) // 128)
                tcs.append(list(range(lo, max(hi, lo + 1))))
            cores.append({"b": b, "j0": j0, "tcs": tcs})
    n0 = max(len(c["tcs"][0]) for c in cores)
    n1 = max(len(c["tcs"][1]) for c in cores)
    key = ("B", n0, n1)
    if key not in _cache:
        _cache[key] = build_kernel_b({"n0": n0, "n1": n1})
    nc = _cache[key]

    # host-side weight prep (norm-weight folding only)
    embp = np.zeros((VP, D), np.float32)
    embp[:V] = embed_W.astype(np.float32)
    embTp = np.ascontiguousarray(embp.T)
    wq_f = np.ascontiguousarray((wq.astype(np.float32)
                                 * qnorm_w.astype(np.float32)[None, :]).T)
    wk_f = np.ascontiguousarray((wk.astype(np.float32)
                                 * kvnorm_w.astype(np.float32)[None, :]).T)
    wv_f = np.ascontiguousarray((wv.astype(np.float32)
                                 * kvnorm_w.astype(np.float32)[None, :]).T)
    wo_f = np.ascontiguousarray(wo.astype(np.float32).T)

    NT = n0 + n1
    in_maps = []
    for c in cores:
        b = c["b"]
        tqr = np.zeros(NT * 256, np.float32)
        pidc = np.full(NT * 128, -1.0, np.float32)  # -1 never matches a j id
        slot = 0
        for jc in range(2):
            lst = c["tcs"][jc]
            # pad each jc segment to its uniform length with repeats of the
            # first chunk (harmless: pid mask kills contributions, and for
            # padded slots we also set pid=-1)
            want = n0 if jc == 0 else n1
            for k_ in range(want):
                if k_ < len(lst):
                    tci = lst[k_]
                    sl = slice(tci * 128, (tci + 1) * 128)
                    tqr[slot * 256:slot * 256 + 128] = tokens[b, sl]
                    tqr[slot * 256 + 128:(slot + 1) * 256] = qtok[b, sl]
                    # rebase pid to local j index (0..255 within this core)
                    pidc[slot * 128:(slot + 1) * 128] = pid[b, sl] - c["j0"]
                slot += 1
        in_maps.append({
            "emb": embp, "embT": embTp, "wqT": wq_f, "wkT": wk_f,
            "wvT": wv_f, "woT": wo_f,
            "tqr": tqr[None, :], "pidc": pidc,
        })
    res = run_bass_kernel_spmd(nc, in_maps, list(range(NCORES)),
                               trace=os.environ.get("KERNEL_TRACE") == "1")
    _cache["tB"] = res.exec_time_ns
    out = np.zeros((B, K, D), np.float32)
    for ci, c in enumerate(cores):
        out[c["b"], c["j0"]:c["j0"] + 256] = res.results[ci]["out"]
    return out


# revision 10
# speedup vs baseline: 1.1696x; 1.1696x over previous
"""Trainium2 kernel for nn_LocalEncoder (BLT-style local encoder).

Key structural insight: every per-token quantity (boundary logit z, rmsnorm
scale, q/k/v projections) depends only on the token ID (vocab=260), so all
dense math runs on the 260-row vocab tables instead of 16384 token rows.
Top-k boundary selection ties (same token id => bit-identical z in the fp32
reference) are broken by index, reproduced exactly on the host.

Pipeline:
  Kernel A (8 cores, DF split 8x384): zv partials = w2_slice @ silu(w1_slice @ embT)
  Host:     zv -> per-row boundary selection (stable by (-z, idx)) -> pos/pid/qtok
  Kernel B (8 cores = 4 seqs x 2 query-halves): one-hot gathers of vocab
            q/k/v tables per token, per-token scores + exp on DVE/ACT,
            block-diagonal softmax via one-hot scatter matmuls, wo proj.
"""

import os
import numpy as np

import concourse.bass as bass
import concourse.bacc as bacc
import concourse.mybir as mybir
from concourse.tile import TileContext
from concourse.alu_op_type import AluOpType
from concourse.bass_utils import run_bass_kernel_spmd
from concourse import masks

F32 = mybir.dt.float32
F32R = mybir.dt.float32r
AFT = mybir.ActivationFunctionType
AX = mybir.AxisListType

B, L, D, V, K, H, HD = 4, 4096, 768, 260, 512, 12, 64
DF = 4 * D
VP = 384          # vocab padded to 3 partition chunks
RMS_EPS = 1e-5
NCORES = 8
FSL = DF // NCORES  # 384 f-rows per core in kernel A

_cache = {}


# --------------------------------------------------------------------------- #
# Kernel A: per-core partial zv over a DF slice (fp32 matmuls for precision)
# --------------------------------------------------------------------------- #
def build_kernel_a():
    nc = bacc.Bacc("TRN2", target_bir_lowering=False, debug=False)
    embT_d = nc.dram_tensor("embT", [D, V], F32, kind="ExternalInput")
    w1T_d = nc.dram_tensor("w1T", [D, FSL], F32, kind="ExternalInput")
    b1_d = nc.dram_tensor("b1s", [FSL], F32, kind="ExternalInput")
    w2_d = nc.dram_tensor("w2s", [FSL], F32, kind="ExternalInput")
    zp_d = nc.dram_tensor("zp", [1, V], F32, kind="ExternalOutput")

    with TileContext(nc) as tc:
        with (
            tc.tile_pool(name="sb", bufs=1) as sb,
            tc.tile_pool(name="ps", bufs=2, space="PSUM") as ps,
        ):
            embT = [sb.tile([128, V], F32, tag=f"embT{d}", name=f"embT{d}") for d in range(6)]
            w1T = [sb.tile([128, FSL], F32, tag=f"w1T{d}", name=f"w1T{d}") for d in range(6)]
            for d in range(6):
                nc.sync.dma_start(embT[d][:, :], embT_d[128 * d:128 * (d + 1), :])
                nc.sync.dma_start(w1T[d][:, :], w1T_d[128 * d:128 * (d + 1), :])
            b1c = sb.tile([128, 3], F32, tag="b1c")
            w2c = sb.tile([128, 3], F32, tag="w2c")
            nc.sync.dma_start(b1c[:, :], b1_d.rearrange("(i p) -> p i", p=128))
            nc.sync.dma_start(w2c[:, :], w2_d.rearrange("(i p) -> p i", p=128))

            zp_ps = ps.tile([1, V], F32, tag="zp")
            for fi in range(3):
                y1p = ps.tile([128, V], F32, tag="y1")
                for d in range(6):
                    nc.tensor.matmul(
                        y1p[:, :], w1T[d][:, 128 * fi:128 * (fi + 1)], embT[d][:, :],
                        start=(d == 0), stop=(d == 5),
                    )
                y1b = sb.tile([128, V], F32, tag="y1b")
                nc.vector.tensor_scalar(y1b[:, :], y1p[:, :], b1c[:, fi:fi + 1],
                                        None, AluOpType.add)
                sig = sb.tile([128, V], F32, tag="sig")
                nc.scalar.activation(sig[:, :], y1b[:, :], AFT.Sigmoid)
                y1s = sb.tile([128, V], F32, tag="y1s")
                nc.vector.tensor_tensor(y1s[:, :], y1b[:, :], sig[:, :],
                                        AluOpType.mult)
                nc.tensor.matmul(zp_ps[:, :], w2c[:, fi:fi + 1], y1s[:, :],
                                 start=(fi == 0), stop=(fi == 2))
            zp_s = sb.tile([1, V], F32, tag="zps")
            nc.vector.tensor_copy(zp_s[:, :], zp_ps[:, :])
            nc.sync.dma_start(zp_d[:, :], zp_s[:, :])
    nc.compile()
    return nc


def run_kernel_a(inputs):
    if "A" not in _cache:
        _cache["A"] = build_kernel_a()
    nc = _cache["A"]
    embT = np.ascontiguousarray(inputs["embed_W"].astype(np.float32).T)
    w1 = inputs["bp_w1"].astype(np.float32)
    b1 = inputs["bp_b1"].astype(np.float32)
    w2 = inputs["bp_w2"].astype(np.float32)[0]
    in_maps = []
    for c in range(NCORES):
        sl = slice(c * FSL, (c + 1) * FSL)
        in_maps.append({
            "embT": embT,
            "w1T": np.ascontiguousarray(w1[sl].T),
            "b1s": np.ascontiguousarray(b1[sl]),
            "w2s": np.ascontiguousarray(w2[sl]),
        })
    res = run_bass_kernel_spmd(nc, in_maps, list(range(NCORES)),
                               trace=os.environ.get("KERNEL_TRACE") == "1")
    _cache["tA"] = res.exec_time_ns
    zv = np.zeros(V, np.float64)
    for c in range(NCORES):
        zv += res.results[c]["zp"][0].astype(np.float64)
    zv += inputs["bp_b2"].astype(np.float64)[0]
    return zv.astype(np.float32)


# --------------------------------------------------------------------------- #
# Host boundary logic
# --------------------------------------------------------------------------- #
def boundary_plan(zv, tokens):
    """Reproduce reference top-k (stable ties by index) + patch structure."""
    zt = zv[tokens]  # [B, L]
    pos = np.zeros((B, K), np.int64)
    for b in range(B):
        key = zt[b].astype(np.float64).copy()
        key[0] = np.inf  # position 0 forced boundary (logprob set to 0 = max)
        order = np.lexsort((np.arange(L), -key))
        pos[b] = np.sort(order[:K])
    pid = (pos[:, None, :] <= np.arange(L)[None, :, None]).sum(-1) - 1  # [B, L]
    qtok = np.take_along_axis(tokens, np.take_along_axis(pos, pid, 1), 1)  # [B, L]
    return pos, pid, qtok


# --------------------------------------------------------------------------- #
# Kernel B: sparse cross-attention via vocab tables + one-hot matmuls
# --------------------------------------------------------------------------- #
def build_kernel_b(jobs):
    """jobs: per-core dict with j0 and per-jc tchunk ranges (python ints ->
    data-dependent instruction stream; same NEFF runs on all 8 cores with the
    max structure, masking handles core differences).  To keep one NEFF for
    all cores, we use the UNION structure: every core runs the same tchunk
    count per jc slot; tchunk indices and j0 are per-core DATA (iota bases
    must be static though) -- so instead we compile per-core variants only if
    structure differs.  Simpler: compile ONE program parameterized by the max
    chunk counts; per-core tchunk starts enter via DRAM-provided pid/tok/qtok
    columns (already per-core rebased by host).
    """
    n0, n1 = jobs["n0"], jobs["n1"]  # tchunks for jc0 / jc1 (uniform, padded)
    nc = bacc.Bacc("TRN2", target_bir_lowering=False, debug=False)

    # vocab tables
    emb_d = nc.dram_tensor("emb", [VP, D], F32, kind="ExternalInput")
    embT_d = nc.dram_tensor("embT", [D, VP], F32R, kind="ExternalInput")
    wqT_d = nc.dram_tensor("wqT", [D, D], F32R, kind="ExternalInput")
    wkT_d = nc.dram_tensor("wkT", [D, D], F32R, kind="ExternalInput")
    wvT_d = nc.dram_tensor("wvT", [D, D], F32R, kind="ExternalInput")
    woT_d = nc.dram_tensor("woT", [D, D], F32R, kind="ExternalInput")
    # per-core token structure, already sliced/padded by host:
    # rows: [1, NT*128] token ids / boundary-token ids (f32), NT = n0+n1
    NT = n0 + n1
    tqr_d = nc.dram_tensor("tqr", [1, NT * 256], F32R, kind="ExternalInput")
    pidc_d = nc.dram_tensor("pidc", [NT * 128], F32, kind="ExternalInput")
    out_d = nc.dram_tensor("out", [256, D], F32, kind="ExternalOutput")

    with TileContext(nc) as tc:
        with (
            tc.tile_pool(name="sb", bufs=1) as sb,
            tc.tile_pool(name="wk", bufs=1) as wkp,
            tc.tile_pool(name="ps", bufs=1, space="PSUM") as ps,
            tc.tile_pool(name="acc", bufs=1, space="PSUM") as accp,
        ):
            # ---- global small tiles ----
            ones_f = sb.tile([1, 128], F32, tag="onesf")
            nc.gpsimd.memset(ones_f[:, :], 1.0)
            ones_r = sb.tile([1, 128], F32R, tag="onesr")
            nc.vector.tensor_copy(ones_r[:, :], ones_f[:, :])
            ident = sb.tile([128, 128], F32, tag="ident")
            masks.make_identity(nc, ident[:, :])
            iotav = sb.tile([128, 3], F32, tag="iotav")  # col p+0/128/256
            for vc in range(3):
                nc.gpsimd.iota(iotav[:, vc:vc + 1], [[0, 1]], base=128 * vc,
                               channel_multiplier=1,
                               allow_small_or_imprecise_dtypes=True)
            iotaj = [sb.tile([128, 128], F32, tag=f"iotaj{jc}", name=f"iotaj{jc}") for jc in range(2)]
            for jc in range(2):
                # value = j0 + jc*128 + f ; j0 enters via host-rebased pid
                nc.gpsimd.iota(iotaj[jc][:, :], [[1, 128]], base=128 * jc,
                               channel_multiplier=0,
                               allow_small_or_imprecise_dtypes=True)

            # ---- load weights / tables ----
            emb = [sb.tile([128, D], F32, tag=f"emb{v}", name=f"emb{v}") for v in range(3)]
            for v in range(3):
                nc.sync.dma_start(emb[v][:, :], emb_d[128 * v:128 * (v + 1), :])
            embT = [sb.tile([128, VP], F32R, tag=f"embT{d}", name=f"embTb{d}") for d in range(6)]
            for d in range(6):
                nc.sync.dma_start(embT[d][:, :], embT_d[128 * d:128 * (d + 1), :])
            wts = {}
            for nm, dd in (("wq", wqT_d), ("wk", wkT_d), ("wv", wvT_d), ("wo", woT_d)):
                wts[nm] = [wkp.tile([128, D], F32R, tag=f"{nm}{d}", name=f"{nm}_{d}") for d in range(6)]
                for d in range(6):
                    nc.sync.dma_start(wts[nm][d][:, :], dd[128 * d:128 * (d + 1), :])

            # per-tchunk structure columns
            pidc = sb.tile([128, NT], F32, tag="pidc")
            nc.sync.dma_start(pidc[:, :], pidc_d.rearrange("(i p) -> p i", p=128))

            # ---- rmsnorm scales: rv (k/v), rv8 (q, includes /8) ----
            msq = sb.tile([128, 3], F32, tag="msq")
            sqjunk = sb.tile([128, D], F32, tag="sqjunk")
            for v in range(3):
                nc.scalar.activation(sqjunk[:, :], emb[v][:, :], AFT.Square,
                                     accum_out=msq[:, v:v + 1])
            # rv = (msq/768 + eps)^-1/2 = exp(-0.5*ln(msq/768 + eps))
            epsc = sb.tile([128, 1], F32, tag="epsc")
            nc.gpsimd.memset(epsc[:, :], RMS_EPS)
            lnv = sb.tile([128, 3], F32, tag="lnv")
            nc.scalar.activation(lnv[:, :], msq[:, :], AFT.Ln,
                                 scale=1.0 / D, bias=epsc[:, :1])
            rv = sb.tile([128, 3], F32, tag="rv")
            nc.scalar.activation(rv[:, :], lnv[:, :], AFT.Exp, scale=-0.5)
            rv8 = sb.tile([128, 3], F32, tag="rv8")
            nc.vector.tensor_scalar(rv8[:, :], rv[:, :], 0.125, None, AluOpType.mult)

            # ---- vocab tables q_s / k_n / v_n [3][128, D] f32r ----
            tabs = {}
            for nm, wname, scl in (("q", "wq", rv8), ("k", "wk", rv), ("v", "wv", rv)):
                tabs[nm] = []
                for v in range(3):
                    tp = ps.tile([128, D], F32, tag="qg", name="tp")
                    for d in range(6):
                        nc.tensor.matmul(
                            tp[:, :512], embT[d][:, 128 * v:128 * (v + 1)],
                            wts[wname][d][:, :512], start=(d == 0), stop=(d == 5))
                        nc.tensor.matmul(
                            tp[:, 512:], embT[d][:, 128 * v:128 * (v + 1)],
                            wts[wname][d][:, 512:], start=(d == 0), stop=(d == 5))
                    ts_ = sb.tile([128, D], F32R, tag=f"tab{nm}{v}")
                    nc.vector.tensor_scalar(ts_[:, :], tp[:, :], scl[:, v:v + 1],
                                            None, AluOpType.mult)
                    tabs[nm].append(ts_)

            # ---- main loop: two query chunks ----
            for jc in range(2):
                ntc = n0 if jc == 0 else n1
                base = 0 if jc == 0 else n0
                acc = accp.tile([128, 1536], F32, tag="acc", name="acc")
                for i in range(ntc):
                    tci = base + i
                    # broadcast token+qtok rows across partitions (one matmul)
                    tq_s = sb.tile([1, 256], F32R, tag="tokslice", name="tq_s", bufs=3)
                    nc.sync.dma_start(tq_s[:, :], tqr_d[:, 256 * tci:256 * (tci + 1)])
                    btok2 = ps.tile([128, 256], F32, tag="btok", name="btok2")
                    nc.tensor.matmul(btok2[:, :], ones_r[:, :], tq_s[:, :],
                                     start=True, stop=True)
                    btok = btok2[:, :128]
                    bqtok = btok2[:, 128:]
                    ohk = []
                    ohq = []
                    for v in range(3):
                        o1 = sb.tile([128, 128], F32R, tag=f"ohk{v}", name=f"o1_{v}", bufs=2)
                        nc.vector.tensor_scalar(o1[:, :], btok,
                                                iotav[:, v:v + 1], None,
                                                AluOpType.is_equal)
                        ohk.append(o1)
                        o2 = sb.tile([128, 128], F32R, tag=f"ohq{v}", name=f"o2_{v}", bufs=2)
                        nc.vector.tensor_scalar(o2[:, :], bqtok,
                                                iotav[:, v:v + 1], None,
                                                AluOpType.is_equal)
                        ohq.append(o2)
                    # gathers: qg/kg/vg [t,768]
                    qg = ps.tile([128, D], F32, tag="qg")
                    kg = ps.tile([128, D], F32, tag="kg")
                    for v in range(3):
                        nc.tensor.matmul(qg[:, :512], ohq[v][:, :],
                                         tabs["q"][v][:, :512],
                                         start=(v == 0), stop=(v == 2))
                        nc.tensor.matmul(qg[:, 512:], ohq[v][:, :],
                                         tabs["q"][v][:, 512:],
                                         start=(v == 0), stop=(v == 2))
                        nc.tensor.matmul(kg[:, :512], ohk[v][:, :],
                                         tabs["k"][v][:, :512],
                                         start=(v == 0), stop=(v == 2))
                        nc.tensor.matmul(kg[:, 512:], ohk[v][:, :],
                                         tabs["k"][v][:, 512:],
                                         start=(v == 0), stop=(v == 2))
                    # scores + exp
                    kgs = sb.tile([128, D], F32, tag="kgs", bufs=2)
                    nc.scalar.copy(kgs[:, :], kg[:, :])
                    prod = sb.tile([128, D], F32, tag="prod", bufs=2)
                    nc.vector.tensor_tensor(prod[:, :], qg[:, :], kgs[:, :],
                                            AluOpType.mult)
                    s12 = sb.tile([128, H], F32, tag="s12", bufs=2)
                    nc.vector.tensor_reduce(
                        ap3(s12, H, 1), ap3(prod, H, HD), AX.X, AluOpType.add)
                    e12f = sb.tile([128, H], F32, tag="e12f", bufs=2)
                    nc.scalar.activation(e12f[:, :], s12[:, :], AFT.Exp)
                    e12 = sb.tile([128, H], F32R, tag="e12", bufs=2)
                    nc.vector.tensor_copy(e12[:, :], e12f[:, :])
                    # value gather (reuses qg slot) and weight
                    vg = ps.tile([128, D], F32, tag="qg")
                    for v in range(3):
                        nc.tensor.matmul(vg[:, :512], ohk[v][:, :],
                                         tabs["v"][v][:, :512],
                                         start=(v == 0), stop=(v == 2))
                        nc.tensor.matmul(vg[:, 512:], ohk[v][:, :],
                                         tabs["v"][v][:, 512:],
                                         start=(v == 0), stop=(v == 2))
                    wv = sb.tile([128, D], F32R, tag="wv", bufs=2)
                    nc.vector.tensor_tensor(ap3(wv, H, HD),
                                            bcast3(e12f, H, HD),
                                            ap3(vg, H, HD), AluOpType.mult)
                    # membership MT [t, j] and scatter
                    mt = sb.tile([128, 128], F32R, tag="mt", bufs=2)
                    nc.vector.tensor_scalar(mt[:, :], iotaj[jc][:, :],
                                            pidc[:, tci:tci + 1], None,
                                            AluOpType.is_equal)
                    nc.tensor.matmul(acc[:, :512], mt[:, :], wv[:, :512],
                                     start=(i == 0), stop=(i == ntc - 1))
                    nc.tensor.matmul(acc[:, 512:768], mt[:, :], wv[:, 512:],
                                     start=(i == 0), stop=(i == ntc - 1))
                    nc.tensor.matmul(acc[:, 1024:1036], mt[:, :], e12[:, :],
                                     start=(i == 0), stop=(i == ntc - 1))
                # ---- finalize jc ----
                lnz = sb.tile([128, H], F32, tag="lnz")
                nc.scalar.activation(lnz[:, :], acc[:, 1024:1036], AFT.Ln)
                zrec = sb.tile([128, H], F32, tag="zrec")
                nc.scalar.activation(zrec[:, :], lnz[:, :], AFT.Exp, scale=-1.0)
                pr = sb.tile([128, D], F32, tag="pr")
                nc.vector.tensor_tensor(ap3(pr, H, HD), bcast3(zrec, H, HD),
                                        ap3(acc, H, HD, width=780), AluOpType.mult)
                fin = ps.tile([128, D], F32, tag="kg", name="fin")
                for d in range(6):
                    trp = ps.tile([128, 128], F32, tag="btok")
                    nc.tensor.transpose(trp[:, :], pr[:, 128 * d:128 * (d + 1)],
                                        ident[:, :])
                    trs = sb.tile([128, 128], F32R, tag="trs", bufs=2)
                    nc.vector.tensor_copy(trs[:, :], trp[:, :])
                    nc.tensor.matmul(fin[:, :512], trs[:, :], wts["wo"][d][:, :512],
                                     start=(d == 0), stop=(d == 5))
                    nc.tensor.matmul(fin[:, 512:], trs[:, :], wts["wo"][d][:, 512:],
                                     start=(d == 0), stop=(d == 5))
                fin_s = sb.tile([128, D], F32, tag="fins")
                nc.vector.tensor_copy(fin_s[:, :], fin[:, :])
                nc.sync.dma_start(out_d[128 * jc:128 * (jc + 1), :], fin_s[:, :])
    nc.compile()
    return nc


def ap3(tile, n, w, width=None):
    """[128, n*w] tile viewed as [128, n, w] (first n*w cols)."""
    p = tile.ap[0] if hasattr(tile, "ap") else None
    t = tile[:, :]
    ps, fs = t.ap[0], t.ap[1]
    return bass.AP(t.tensor, t.offset, [list(ps), [fs[0] * w, n], [fs[0], w]])


def bcast3(tile, n, w):
    """[128, n] tile broadcast to [128, n, w] via 0-stride inner dim."""
    t = tile[:, :]
    ps, fs = t.ap[0], t.ap[1]
    return bass.AP(t.tensor, t.offset, [list(ps), [fs[0], n], [0, w]])


# --------------------------------------------------------------------------- #
# top-level
# --------------------------------------------------------------------------- #
def kernel(tokens, embed_W, bp_w1, bp_b1, bp_w2, bp_b2, wq, wk, wv, wo,
           qnorm_w, kvnorm_w, k_patches):
    tokens = np.asarray(tokens).astype(np.int64)
    inputs = dict(tokens=tokens, embed_W=embed_W, bp_w1=bp_w1, bp_b1=bp_b1,
                  bp_w2=bp_w2, bp_b2=bp_b2)
    zv = run_kernel_a(inputs)
    pos, pid, qtok = boundary_plan(zv, tokens)

    # per-core job structure: core = 2*b + half; queries [half*256, half*256+256)
    cores = []
    for b in range(B):
        for half in range(2):
            j0 = half * 256
            ends = [pos[b, j0 + 128] if j0 + 128 < K else L,
                    pos[b, j0 + 256] if j0 + 256 < K else L]
            starts = [pos[b, j0], pos[b, j0 + 128] if j0 + 128 < K else L]
            tcs = []
            for jc in range(2):
                lo, hi = int(starts[jc]) // 128, -(-int(ends[jc]) // 128)
                tcs.append(list(range(lo, max(hi, lo + 1))))
            cores.append({"b": b, "j0": j0, "tcs": tcs})
    n0 = max(len(c["tcs"][0]) for c in cores)
    n1 = max(len(c["tcs"][1]) for c in cores)
    key = ("B", n0, n1)
    if key not in _cache:
        _cache[key] = build_kernel_b({"n0": n0, "n1": n1})
    nc = _cache[key]

    # host-side weight prep (norm-weight folding only)
    embp = np.zeros((VP, D), np.float32)
    embp[:V] = embed_W.astype(np.float32)
    embTp = np.ascontiguousarray(embp.T)
    wq_f = np.ascontiguousarray((wq.astype(np.float32)
                                 * qnorm_w.astype(np.float32)[None, :]).T)
    wk_f = np.ascontiguousarray((wk.astype(np.float32)
                                 * kvnorm_w.astype(np.float32)[None, :]).T)
    wv_f = np.ascontiguousarray((wv.astype(np.float32)
                                 * kvnorm_w.astype(np.float32)[None, :]).T)
    wo_f = np.ascontiguousarray(wo.astype(np.float32).T)

    NT = n0 + n1
    in_maps = []
    for c in cores:
        b = c["b"]
        tqr = np.zeros(NT * 256, np.float32)
        pidc = np.full(NT * 128, -1.0, np.float32)  # -1 never matches a j id
        slot = 0
        for jc in range(2):
            lst = c["tcs"][jc]
            # pad each jc segment to its uniform length with repeats of the
            # first chunk (harmless: pid mask kills contributions, and for
            # padded slots we also set pid=-1)
            want = n0 if jc == 0 else n1
            for k_ in range(want):
                if k_ < len(lst):
                    tci = lst[k_]
                    sl = slice(tci * 128, (tci + 1) * 128)
                    tqr[slot * 256:slot * 256 + 128] = tokens[b, sl]
                    tqr[slot * 256 + 128:(slot + 1) * 256] = qtok[b, sl]
                    # rebase pid to local j index (0..255 within this core)
                    pidc[slot * 128:(slot + 1) * 128] = pid[b, sl] - c["j0"]
                slot += 1
        in_maps.append({
            "emb": embp, "embT": embTp, "wqT": wq_f, "wkT": wk_f,
            "wvT": wv_f, "woT": wo_f,
            "tqr": tqr[None, :], "pidc": pidc,
        })
    res = run_bass_kernel_spmd(nc, in_maps, list(range(NCORES)),
                               trace=os.environ.get("KERNEL_TRACE") == "1")
    _cache["tB"] = res.exec_time_ns
    out = np.zeros((B, K, D), np.float32)
    for ci, c in enumerate(cores):
        out[c["b"], c["j0"]:c["j0"] + 256] = res.results[ci]["out"]
    return out
